# revision 1
# baseline (speedup 1.0000x reference)
"""Self-contained TRN2 Bass kernel for nn_GATRotationRegressor.

kernel(**inputs) -> [16384, 24, 6] fp32. Data-parallel over 8 NeuronCores;
all layouts/shapes hardcoded for B=16384, J=24, H=128, heads=4, L=3.
"""
from contextlib import ExitStack

import numpy as np

import concourse.bass as bass
import concourse.tile as tile
from concourse import mybir





PARENTS = [-1, 0, 0, 0, 1, 2, 3, 4, 5, 6, 7, 8, 9, 9, 9, 12, 13, 14, 16, 17, 18, 19, 20, 21]
B, J, IN_DIM, H, HEADS, OUT_DIM, L = 16384, 24, 3, 128, 4, 6, 3
C = H // HEADS
SLOPE = 0.2
KMAX = 5           # padded neighbor slots per dst
# feature permutation: device feature n = c*4 + h  <->  model feature o = h*32 + c
FPERM_O_OF_N = np.array([(n % HEADS) * C + n // HEADS for n in range(H)])
N_CORES = 8
BC = B // N_CORES  # graphs per core


def children(j):
    return [c for c, p in enumerate(PARENTS) if p == j]


def edge_slots():
    """For every real edge (src, dst) return its slot k at dst."""
    slots = {}
    for j in range(J):
        slots[(j, j)] = 0
        p = PARENTS[j]
        if p >= 0:
            slots[(p, j)] = 1
        for i, c in enumerate(children(j)):
            slots[(c, j)] = 2 + i
    return slots


def build_runs():
    """Greedy decomposition of the 70 edges into strided runs.

    Returns list of (src0, sstep, dst0, n, slot). dst steps by +1 within a
    run except the self run (both step 1) and const-src runs (sstep=0).
    A run requires: slot constant, dst strictly increasing by 1 (so output
    elements are distinct), src stepping by a constant (0 or 1).
    """
    slots = edge_slots()
    edges = sorted(slots.keys(), key=lambda e: (slots[e], e[1]))  # by (slot, dst)
    runs = []
    used = set()
    for e in edges:
        if e in used:
            continue
        src, dst = e
        k = slots[e]
        for sstep in (1, 0):
            n = 1
            while True:
                nxt = (src + sstep * n, dst + n)
                if nxt in slots and slots[nxt] == k and nxt not in used:
                    n += 1
                else:
                    break
            if n > 1 or sstep == 0:
                break
        for i in range(n):
            used.add((src + sstep * i, dst + i))
        runs.append((src, sstep, dst, n, k))
    assert sum(r[3] for r in runs) == 70, sum(r[3] for r in runs)
    return runs


def _edges():
    e = []
    for c, p in enumerate(PARENTS):
        if p >= 0:
            e.append((p, c)); e.append((c, p))
    for j in range(len(PARENTS)):
        e.append((j, j))
    a = np.asarray(e, dtype=np.int32)
    return a[:, 0], a[:, 1]


def _ln_np(x, g, b, eps=1e-5):
    m = x.mean(-1, keepdims=True)
    v = x.var(-1, keepdims=True)
    return (x - m) / np.sqrt(v + eps) * g + b


def np_reference(x, p, collect=None):
    """Numpy port of reference.py for an arbitrary batch. p: raw input dict.
    collect: optional dict to stash intermediates for stage debugging."""
    Bn = x.shape[0]
    src, dst = _edges()
    h = x @ p["in_w"] + p["in_b"] + p["pos"][None]
    res = x @ p["res_w"] + p["res_b"]
    cc = collect if collect is not None else {}
    for l in range(L):
        hp = h
        xl = (h @ p["gat_w"][l]).reshape(Bn, J, HEADS, C)
        a_s = np.einsum('bjhc,hc->bjh', xl, p["att_s"][l])
        a_d = np.einsum('bjhc,hc->bjh', xl, p["att_d"][l])
        cc[f"xl{l}"] = xl; cc[f"a_s{l}"] = a_s; cc[f"a_d{l}"] = a_d
        e = a_s[:, src] + a_d[:, dst]
        e = np.where(e > 0, e, SLOPE * e)
        ex = np.exp(e)                       # no max-subtraction (values small)
        den = np.zeros((Bn, J, HEADS), e.dtype)
        np.add.at(den, (slice(None), dst), ex)
        alpha = ex / den[:, dst]
        cc[f"alpha{l}"] = alpha
        msg = xl[:, src] * alpha[..., None]
        out = np.zeros_like(xl)
        np.add.at(out, (slice(None), dst), msg)
        out = out.reshape(Bn, J, H) + p["gat_b"][l]
        cc[f"agg{l}"] = out
        out = np.where(out > 0, out, np.exp(np.minimum(out, 0)) - 1)  # elu
        out = _ln_np(out, p["ln_g"][l], p["ln_b"][l])
        cc[f"o{l}"] = out
        h = out + hp if l > 0 else out
    h = h + res
    y = np.maximum(h @ p["w1"] + p["b1"], 0)
    y = _ln_np(y, p["lng2"], p["lnb2"])
    return y @ p["w2"] + p["b2"]


def host_prep(inputs):
    """All host-side weight algebra. Returns dict of device tensors (shared
    across cores) + per-core x_t layout builder."""
    f32 = np.float32
    in_w, in_b = np.asarray(inputs["in_w"]), np.asarray(inputs["in_b"])
    res_w, res_b = np.asarray(inputs["res_w"]), np.asarray(inputs["res_b"])
    pos = np.asarray(inputs["pos"])
    gat_w = np.asarray(inputs["gat_w"])
    att_s, att_d = np.asarray(inputs["att_s"]), np.asarray(inputs["att_d"])
    gat_b = np.asarray(inputs["gat_b"])
    ln_g, ln_b = np.asarray(inputs["ln_g"]), np.asarray(inputs["ln_b"])
    w1, b1 = np.asarray(inputs["w1"]), np.asarray(inputs["b1"])
    lng2, lnb2 = np.asarray(inputs["lng2"]), np.asarray(inputs["lnb2"])
    w2, b2 = np.asarray(inputs["w2"]), np.asarray(inputs["b2"])

    # per-layer score projections: ws[l] [H, 8] cols = (a_s h0..h3, a_d h0..h3)
    ws = np.zeros((L, H, 8), f32)
    for l in range(L):
        for h in range(HEADS):
            ws[l, :, h] = gat_w[l, :, h * C:(h + 1) * C] @ att_s[l, h]
            ws[l, :, 4 + h] = gat_w[l, :, h * C:(h + 1) * C] @ att_d[l, h]

    # device feature order: n = c*4 + h (head innermost). Permute every
    # H-indexed axis that faces the device feature space.
    P = FPERM_O_OF_N

    # L0 folded stationaries on rhs28 = [x(3); ones(1); onehot_j(24)]
    W0p = np.zeros((28, H), f32)
    W0p[0:3] = in_w @ gat_w[0]
    W0p[3] = in_b @ gat_w[0]
    W0p[4:28] = pos @ gat_w[0]
    W0p = W0p[:, P]
    WS0p = np.zeros((28, 8), f32)
    WS0p[0:3] = in_w @ ws[0]
    WS0p[3] = in_b @ ws[0]
    WS0p[4:28] = pos @ ws[0]
    # residual fold for w1 pass: rows 0-2 res_w@w1; row 3 res_b@w1 (b1 added at evac)
    gat_w_d = gat_w[:, P][:, :, P]          # rows+cols permuted (l>=1 use)
    ws_d = ws[:, P]                          # rows permuted
    gat_b_d = gat_b[:, P]
    ln_g_d = ln_g[:, P]
    ln_b_d = ln_b[:, P]
    w1_d = w1[P, :]
    RW1p = np.zeros((28, H // 2), f32)
    RW1p[0:3] = res_w @ w1
    RW1p[3] = res_b @ w1
    # final LN2/w2 fold
    W2p = (lng2[:, None] * w2).astype(f32)          # [64, 6]
    c2 = W2p.sum(axis=0)                             # colsum for -mu*r term
    b2p = lnb2 @ w2 + b2                             # [6]

    return dict(
        W0p=W0p, WS0p=WS0p, RW1p=RW1p, W2p=W2p, c2=c2, b2p=b2p,
        gat_w=gat_w_d, ws=ws_d, gat_b=gat_b_d, ln_g=ln_g_d, ln_b=ln_b_d,
        w1=w1_d, b1=b1,
    )


def make_rhs_const(G):
    """Rows 3..27 of rhs28: [ones; onehot_j] as [25, J*G] bf16."""
    import ml_dtypes
    N = J * G
    out = np.zeros((25, N), np.float32)
    out[0] = 1.0
    for j in range(J):
        out[1 + j, j * G:(j + 1) * G] = 1.0
    return out.astype(ml_dtypes.bfloat16)


def make_x_t(x_core, G):
    """x_core [BCk, 24, 3] -> x_t [3, BCk*24] bf16 with col = blk*G*24 + j*G + g."""
    import ml_dtypes
    BCk = x_core.shape[0]
    nblk = BCk // G
    # [blk, g, j, d] -> [d, blk, j, g]
    xt = x_core.reshape(nblk, G, J, IN_DIM).transpose(3, 0, 2, 1).reshape(IN_DIM, BCk * J)
    return xt.astype(ml_dtypes.bfloat16)



F32 = mybir.dt.float32
BF16 = mybir.dt.bfloat16
AF = mybir.ActivationFunctionType
ALU = mybir.AluOpType
AX = mybir.AxisListType

RUNS = build_runs()
CHUNK = 1024          # matmul moving-mode chunk (columns)
MICRO = 128           # stationary-mode micro-chunk (columns)


def rawap(t, off, dims):
    a = t[:]
    return bass.AP(tensor=a.tensor, offset=a.offset + off,
                   ap=[[a.ap[0][0], a.ap[0][1]]] + [list(d) for d in dims])


def emit_rsqrt(nc, out, in_, tmp, tmp2):
    """out = 1/sqrt(in_) via quake init + 2 Newton iters. All [128, F] F32.
    tmp/tmp2 scratch; in_ preserved."""
    I32 = mybir.dt.int32
    ib = in_.bitcast(I32)
    # y0i = 0x5F3759DF - (i >> 1)
    nc.vector.tensor_scalar(tmp.bitcast(I32), ib, 1, None,
                            op0=ALU.logical_shift_right)
    nc.vector.tensor_scalar(tmp.bitcast(I32), tmp.bitcast(I32), -1, 0x5F3759DF,
                            op0=ALU.mult, op1=ALU.add)
    # two Newton iterations: y = y * (1.5 - 0.5 * x * y^2)
    for _ in range(2):
        nc.vector.tensor_tensor(out=tmp2, in0=tmp, in1=tmp, op=ALU.mult)
        nc.vector.tensor_tensor(out=tmp2, in0=tmp2, in1=in_, op=ALU.mult)
        nc.vector.tensor_scalar(tmp2, tmp2, -0.5, 1.5, op0=ALU.mult, op1=ALU.add)
        nc.vector.tensor_tensor(out=tmp, in0=tmp, in1=tmp2, op=ALU.mult)
    nc.vector.tensor_copy(out, tmp)


def kernel_body(ctx, tc, io, G2, n_blocks, dbg_l=None):
    """io: dict name -> bass.AP (dram). Emits the kernel."""
    nc = tc.nc
    G = 128 * G2
    N = J * G
    NB = n_blocks
    NE = N // 2   # elu/e32 chunking

    x_t, y_out = io["x_t"], io.get("y")

    wp = ctx.enter_context(tc.tile_pool(name="wp", bufs=1))
    fm = gm = hp = ep = zp = small = stats = wp
    consts = ctx.enter_context(tc.tile_pool(name="consts", bufs=1))
    psum = ctx.enter_context(tc.tile_pool(name="psum", bufs=2, space="PSUM"))
    psg = ctx.enter_context(tc.tile_pool(name="psg", bufs=2, space="PSUM"))

    # ---- persistent constants ----
    def cload(name, shape, dtype=BF16, src=None):
        t = consts.tile(list(shape), dtype, tag=f"c_{name}")
        nc.sync.dma_start(t[:], src if src is not None else io[name])
        return t

    c_W0p = cload("W0p", (28, H))
    c_WS0p = cload("WS0p", (28, 8))
    c_GW = [cload(f"GW{l}", (H, H), src=io["GW"][l]) for l in (1, 2)]
    c_WS = [cload(f"WSl{l}", (H, 8), src=io["WS"][l]) for l in (1, 2)]
    c_W1 = cload("W1", (H, H // 2))
    c_RW1p = cload("RW1p", (28, H // 2))
    c_W2p = cload("W2p", (H // 2, OUT_DIM))
    c_gb, c_lng, c_lnb = [], [], []
    for l in range(L):
        c_gb.append(cload(f"gb{l}", (H, 1), F32, io["GB"][l].unsqueeze(1)))
        c_lng.append(cload(f"lng{l}", (H, 1), F32, io["LNG"][l].unsqueeze(1)))
        c_lnb.append(cload(f"lnb{l}", (H, 1), F32, io["LNB"][l].unsqueeze(1)))
    c_b1 = cload("b1", (H // 2, 1), F32, io["B1"].unsqueeze(1))
    C2B2 = io["C2B2"]
    c_c2 = cload("c2", (128, OUT_DIM), F32,
                 bass.AP(tensor=C2B2.tensor, offset=C2B2.offset,
                         ap=[[0, 128], [1, OUT_DIM]]))
    c_b2p = cload("b2p", (128, OUT_DIM), F32,
                  bass.AP(tensor=C2B2.tensor, offset=C2B2.offset + OUT_DIM,
                          ap=[[0, 128], [1, OUT_DIM]]))
    c_ones = consts.tile([128, 1], BF16, tag="c_ones")
    nc.vector.memset(c_ones[:], 1.0)

    n_mc = N // MICRO
    jD = G2 * 128
    aD = G2 * HEADS * KMAX
    eD = aD

    def transpose(dst_t, src_t, eng):
        eng.dma_start_transpose(
            dst_t[:].rearrange("p (k q) -> p k q", q=128), src_t[:])

    def ln_smalls(sst, scale):
        """sums [128, (m,2)] -> (sr, sm2) fp32 tiles."""
        smu = stats.tile([128, J * G2], F32, tag="smu")
        svar = stats.tile([128, J * G2], F32, tag="svar")
        sr = stats.tile([128, J * G2], F32, tag="sr")
        sm2 = stats.tile([128, J * G2], F32, tag="sm2")
        t1 = stats.tile([128, J * G2], F32, tag="st1")
        t2 = stats.tile([128, J * G2], F32, tag="st2")
        stv = sst[:].rearrange("p (m s) -> p m s", s=2)
        nc.vector.tensor_scalar_mul(smu[:], stv[:, :, 0], scale)
        nc.vector.tensor_scalar_mul(svar[:], stv[:, :, 1], scale)
        nc.vector.tensor_tensor(out=sm2[:], in0=smu[:], in1=smu[:],
                                op=ALU.mult)
        nc.vector.tensor_tensor(out=svar[:], in0=svar[:], in1=sm2[:],
                                op=ALU.subtract)
        nc.vector.tensor_scalar_add(svar[:], svar[:], 1e-5)
        emit_rsqrt(nc, sr[:], svar[:], t1[:], t2[:])
        nc.vector.tensor_tensor(out=sm2[:], in0=smu[:], in1=sr[:],
                                op=ALU.mult)
        return sr, sm2

    def stat_mms(src_a, src_b, kdim):
        """Stationary-mode per-column sums of src_a/src_b -> sst."""
        sst = stats.tile([128, J * G2 * 2], F32, tag="sst", bufs=2)
        for mc0 in range(0, n_mc, 32):
            gn = min(32, n_mc - mc0)
            pm = psg.tile([128, 128], F32, tag="pt_small")
            for i in range(gn):
                mc = mc0 + i
                nc.tensor.matmul(pm[:, i * 2:i * 2 + 1],
                                 src_a[0:kdim, mc * MICRO:(mc + 1) * MICRO],
                                 c_ones[0:kdim, :], start=True, stop=True)
                nc.tensor.matmul(pm[:, i * 2 + 1:i * 2 + 2],
                                 src_b[0:kdim, mc * MICRO:(mc + 1) * MICRO],
                                 c_ones[0:kdim, :], start=True, stop=True)
            nc.vector.tensor_copy(sst[:, mc0 * 2:(mc0 + gn) * 2],
                                  pm[:, 0:gn * 2])
        return sst

    for blk in range(NB):
        t_h = hp.tile([128, N], BF16, tag="t_h", bufs=2)
        rhs28 = hp.tile([28, N], BF16, tag="rhs28", bufs=2)
        nc.sync.dma_start(rhs28[3:28, :], io["rhs_const"])
        nc.sync.dma_start(rhs28[0:3, :], x_t[:, blk * N:(blk + 1) * N])

        for l in range(L):
            # ---- xl matmul (moving mode) + ACT evac-cast (+gat_b) ----
            t_xlf = fm.tile([128, N], BF16, tag="fmt", bufs=3)
            for c0 in range(0, N, CHUNK):
                pt = psum.tile([128, CHUNK], F32, tag="pt_mm")
                for s0 in range(0, CHUNK, 512):
                    lhs = c_W0p if l == 0 else c_GW[l - 1]
                    rhs = (rhs28 if l == 0 else t_h)[:, c0 + s0:c0 + s0 + 512]
                    nc.tensor.matmul(pt[:, s0:s0 + 512], lhs[:], rhs,
                                     start=True, stop=True)
                nc.scalar.activation(t_xlf[:, c0:c0 + CHUNK], pt[:],
                                     AF.Identity, bias=c_gb[l][:, 0:1],
                                     scale=1.0)

            # ---- scores (stationary mode) -> sS [p, (j, g2, 8)] ----
            sS = small.tile([128, J * G2 * 8], F32, tag="sS", bufs=2)
            for mc0 in range(0, n_mc, 16):
                gn = min(16, n_mc - mc0)
                pm = psg.tile([128, 128], F32, tag="pt_small")
                for i in range(gn):
                    mc = mc0 + i
                    lhs = (rhs28 if l == 0 else t_h)[:, mc * MICRO:(mc + 1) * MICRO]
                    w = c_WS0p if l == 0 else c_WS[l - 1]
                    nc.tensor.matmul(pm[:, i * 8:(i + 1) * 8], lhs, w[:],
                                     start=True, stop=True)
                nc.vector.tensor_copy(sS[:, mc0 * 8:(mc0 + gn) * 8],
                                      pm[:, 0:gn * 8])

            # ---- T1: xl FM -> GM ----
            t_xlg = gm.tile([128, N], BF16, tag="gmt", bufs=3)
            transpose(t_xlg, t_xlf, nc.sync)
            if dbg_l == l and "dbg_sS" in io:
                nc.sync.dma_start(io["dbg_sS"], sS[:])

            # ---- E build (gpsimd) ----
            sE = small.tile([128, J * G2 * HEADS * KMAX], F32, tag="sE")
            nc.vector.memset(sE[:], -10000.0)
            for (src0, sstep, dst0, n, k) in RUNS:
                out_ap = rawap(sE, dst0 * eD + k * HEADS,
                               [(eD, n), (KMAX * HEADS, G2), (1, HEADS)])
                as_ap = rawap(sS, src0 * G2 * 8,
                              [(G2 * 8 * sstep, n), (8, G2), (1, HEADS)])
                ad_ap = rawap(sS, dst0 * G2 * 8 + 4,
                              [(G2 * 8, n), (8, G2), (1, HEADS)])
                nc.gpsimd.tensor_tensor(out=out_ap, in0=as_ap, in1=ad_ap,
                                        op=ALU.add)

            # ---- P = exp(lrelu(E)); den; alpha ----
            sP = small.tile([128, J * G2 * HEADS * KMAX], F32, tag="sP")
            sP2 = small.tile([128, J * G2 * HEADS * KMAX], F32, tag="sP2")
            nc.vector.tensor_scalar(sP[:], sE[:], 0.0, SLOPE, op0=ALU.min,
                                    op1=ALU.mult)
            nc.vector.tensor_scalar_max(sP2[:], sE[:], 0.0)
            nc.vector.tensor_tensor(out=sP[:], in0=sP[:], in1=sP2[:],
                                    op=ALU.add)
            nc.scalar.activation(sP[:], sP[:], AF.Exp)
            sden = small.tile([128, J * G2 * HEADS], F32, tag="sden")
            sdr = small.tile([128, J * G2 * HEADS], F32, tag="sdr")
            sA = small.tile([128, J * G2 * HEADS * KMAX], BF16, tag="sA", bufs=2)
            nc.vector.tensor_reduce(
                out=sden[:].rearrange("p (d g h) -> p d g h", d=J, g=G2),
                in_=rawap(sP, 0, [(eD, J), (KMAX * HEADS, G2), (1, HEADS),
                                  (HEADS, KMAX)]),
                axis=AX.X, op=ALU.add)
            nc.vector.reciprocal(sdr[:], sden[:])
            nc.vector.tensor_tensor(
                out=rawap(sA, 0, [(eD, J), (KMAX * HEADS, G2),
                                  (HEADS, KMAX), (1, HEADS)]),
                in0=rawap(sP, 0, [(eD, J), (KMAX * HEADS, G2),
                                  (HEADS, KMAX), (1, HEADS)]),
                in1=rawap(sdr, 0, [(G2 * HEADS, J), (HEADS, G2),
                                   (0, KMAX), (1, HEADS)]),
                op=ALU.mult)
            if dbg_l == l and "dbg_sA" in io:
                nc.sync.dma_start(io["dbg_sA"], sA[:])
            if dbg_l == l and "dbg_xlg" in io:
                nc.sync.dma_start(io["dbg_xlg"], t_xlg[:])

            # ---- aggregation ----
            t_v = gm.tile([128, N], BF16, tag="gmt", bufs=3)

            def xl_ap(j0, sstep, n):
                return rawap(t_xlg, j0 * jD,
                             [(jD * sstep, n), (128, G2), (HEADS, C),
                              (1, HEADS)])

            def al_ap(dst0, n, k):
                return rawap(sA, dst0 * aD + k * HEADS,
                             [(aD, n), (KMAX * HEADS, G2), (0, C),
                              (1, HEADS)])

            def v_ap(dst0, n, buf):
                return rawap(buf, dst0 * jD,
                             [(jD, n), (128, G2), (HEADS, C), (1, HEADS)])

            for ri, (src0, sstep, dst0, n, k) in enumerate(RUNS):
                if ri == 0:
                    nc.vector.tensor_tensor(out=v_ap(0, 24, t_v),
                                            in0=xl_ap(0, 1, 24),
                                            in1=al_ap(0, 24, 0), op=ALU.mult)
                    continue
                t_tmp = fm.tile([128, N], BF16, tag="atmp", bufs=1)
                nc.vector.tensor_tensor(out=v_ap(dst0, n, t_tmp),
                                        in0=xl_ap(src0, sstep, n),
                                        in1=al_ap(dst0, n, k), op=ALU.mult)
                nc.vector.tensor_tensor(out=v_ap(dst0, n, t_v),
                                        in0=v_ap(dst0, n, t_v),
                                        in1=v_ap(dst0, n, t_tmp), op=ALU.add)
            if dbg_l == l and "dbg_v" in io:
                nc.sync.dma_start(io["dbg_v"], t_v[:])

            # ---- elu: w = relu(v) + expm1(min(v,0)), chunked fp32 expm1 ----
            t_w = gm.tile([128, N], BF16, tag="gmt", bufs=3)
            for c0 in range(0, N, NE):
                t_t = fm.tile([128, NE], BF16, tag="atmp", bufs=1)
                e32 = ep.tile([128, NE], F32, tag="e32", bufs=1)
                nc.vector.tensor_scalar_min(t_t[:], t_v[:, c0:c0 + NE], 0.0)
                nc.scalar.activation(e32[:], t_t[:], AF.Exp)
                nc.vector.tensor_scalar_add(t_t[:], e32[:], -1.0)
                nc.vector.tensor_scalar_max(t_w[:, c0:c0 + NE],
                                            t_v[:, c0:c0 + NE], 0.0)
                nc.vector.tensor_tensor(out=t_w[:, c0:c0 + NE],
                                        in0=t_w[:, c0:c0 + NE],
                                        in1=t_t[:], op=ALU.add)
            if dbg_l == l and "dbg_w" in io:
                nc.sync.dma_start(io["dbg_w"], t_w[:])

            # ---- T2 + w^2 (gpsimd) ----
            t_wf = fm.tile([128, N], BF16, tag="fmt", bufs=3)
            transpose(t_wf, t_w, nc.scalar)
            t_w2 = fm.tile([128, N], BF16, tag="fmt", bufs=3)
            nc.gpsimd.tensor_tensor(out=t_w2[:], in0=t_wf[:], in1=t_wf[:],
                                    op=ALU.mult)

            # ---- LN stats + smalls ----
            sst = stat_mms(t_wf, t_w2, 128)
            sr, sm2 = ln_smalls(sst, 1.0 / H)

            # ---- LNraw = w*r - m2 (mult on gpsimd, sub on vector) ----
            t_lnr = gm.tile([128, N], BF16, tag="gmt", bufs=3)
            rb = sr[:].rearrange("p (j g) -> p j g", j=J).unsqueeze(3) \
                .broadcast_to((128, J, G2, H))
            m2b = sm2[:].rearrange("p (j g) -> p j g", j=J).unsqueeze(3) \
                .broadcast_to((128, J, G2, H))
            def gm3(t):
                return t[:].rearrange("p (j g h) -> p j g h", j=J, g=G2)
            nc.gpsimd.tensor_tensor(out=gm3(t_lnr), in0=gm3(t_w), in1=rb,
                                    op=ALU.mult)
            nc.vector.tensor_tensor(out=gm3(t_lnr), in0=gm3(t_lnr), in1=m2b,
                                    op=ALU.subtract)

            # ---- T3; O = LNraw*g + b (ACT); h += O ----
            t_o = fm.tile([128, N], BF16, tag="fmt", bufs=3)
            transpose(t_o, t_lnr, nc.sync)
            dst_t = t_h if l == 0 else t_o
            nc.scalar.activation(dst_t[:], t_o[:], AF.Identity,
                                 bias=c_lnb[l][:, 0:1], scale=c_lng[l][:, 0:1])
            if l > 0:
                nc.vector.tensor_tensor(out=t_h[:], in0=t_h[:], in1=t_o[:],
                                        op=ALU.add)
            if dbg_l == l and "dbg_h" in io:
                nc.sync.dma_start(io["dbg_h"], t_h[:])

        # ================= final MLP =================
        t_z = zp.tile([64, N], BF16, tag="t_z", bufs=1)
        for c0 in range(0, N, CHUNK):
            pt = psum.tile([128, CHUNK], F32, tag="pt_mm")
            for s0 in range(0, CHUNK, 512):
                nc.tensor.matmul(pt[0:64, s0:s0 + 512], c_W1[:],
                                 t_h[:, c0 + s0:c0 + s0 + 512],
                                 start=True, stop=False)
                nc.tensor.matmul(pt[0:64, s0:s0 + 512], c_RW1p[:],
                                 rhs28[:, c0 + s0:c0 + s0 + 512],
                                 start=False, stop=True)
            nc.vector.tensor_scalar(t_z[:, c0:c0 + CHUNK], pt[0:64, :],
                                    c_b1[:, 0:1], 0.0, op0=ALU.add,
                                    op1=ALU.max)
        t_z2 = fm.tile([64, N], BF16, tag="atmp", bufs=1)
        nc.gpsimd.tensor_tensor(out=t_z2[:], in0=t_z[:], in1=t_z[:],
                                op=ALU.mult)
        sst = stat_mms(t_z, t_z2, 64)
        sr, sm2 = ln_smalls(sst, 2.0 / H)

        st6 = small.tile([128, J * G2 * OUT_DIM], F32, tag="st6")
        for mc0 in range(0, n_mc, 16):
            gn = min(16, n_mc - mc0)
            pm = psg.tile([128, 128], F32, tag="pt_small")
            for i in range(gn):
                mc = mc0 + i
                nc.tensor.matmul(pm[:, i * OUT_DIM:(i + 1) * OUT_DIM],
                                 t_z[:, mc * MICRO:(mc + 1) * MICRO],
                                 c_W2p[:], start=True, stop=True)
            nc.vector.tensor_copy(st6[:, mc0 * OUT_DIM:(mc0 + gn) * OUT_DIM],
                                  pm[:, 0:gn * OUT_DIM])

        sy = small.tile([128, J * G2 * OUT_DIM], F32, tag="sy")
        t6v = st6[:].rearrange("p (j g o) -> p j g o", j=J, g=G2)
        yv = sy[:].rearrange("p (j g o) -> p j g o", j=J, g=G2)
        rb = sr[:].rearrange("p (j g) -> p j g", j=J).unsqueeze(3) \
            .broadcast_to((128, J, G2, OUT_DIM))
        m2b = sm2[:].rearrange("p (j g) -> p j g", j=J).unsqueeze(3) \
            .broadcast_to((128, J, G2, OUT_DIM))
        c2b = rawap(c_c2, 0, [(0, J), (0, G2), (1, OUT_DIM)])
        b2b = rawap(c_b2p, 0, [(0, J), (0, G2), (1, OUT_DIM)])
        nc.vector.tensor_tensor(out=yv, in0=t6v, in1=rb, op=ALU.mult)
        nc.vector.tensor_tensor(out=t6v, in0=m2b, in1=c2b, op=ALU.mult)
        nc.vector.tensor_tensor(out=yv, in0=yv, in1=t6v, op=ALU.subtract)
        nc.vector.tensor_tensor(out=yv, in0=yv, in1=b2b, op=ALU.add)

        if y_out is None:
            continue
        yo = bass.AP(
            tensor=y_out.tensor,
            offset=y_out.offset + blk * G * J * OUT_DIM,
            ap=[[J * OUT_DIM, 128], [OUT_DIM, J], [128 * J * OUT_DIM, G2],
                [1, OUT_DIM]])
        nc.sync.dma_start(yo, sy[:].rearrange("p (j g o) -> p j g o", j=J, g=G2))


# ======================================================================
# Host driver: kernel(**inputs) -> np.ndarray
# ======================================================================
N_CORES = 8
G2_FULL = 2
NB_FULL = (B // N_CORES) // (128 * G2_FULL)


def _install_ntff_shim():
    import sys, types
    if "antenv.axon_hooks" in sys.modules:
        return
    mod = types.ModuleType("antenv.axon_hooks")
    mod._hook = None
    mod.set_axon_ntff_profile_hook = lambda h: setattr(mod, "_hook", h)
    mod.get_axon_ntff_profile_hook = lambda: mod._hook
    sys.modules["antenv.axon_hooks"] = mod
    try:
        from trn_agent_boot.trn_boot import _ntff_profile_via_ctypes
        mod.set_axon_ntff_profile_hook(
            _ntff_profile_via_ctypes("/opt/axon/libaxon_pjrt.so"))
    except Exception:
        pass
    try:
        import concourse.bass_utils as bu
        bu.upload_artifacts = lambda tmpdir: tmpdir
    except Exception:
        pass


_NC_CACHE = {}


def _build_nc(G2, NB):
    key = (G2, NB)
    if key in _NC_CACHE:
        return _NC_CACHE[key]
    from contextlib import ExitStack
    from concourse import bacc
    nc = bacc.Bacc("TRN2", target_bir_lowering=False, debug=False,
                   num_devices=N_CORES)
    G = 128 * G2
    BCk = G * NB
    dt = nc.dram_tensor
    io = {}
    def din(name, shape, dtype):
        io[name] = dt(name, shape, dtype, kind="ExternalInput").ap()
    din("x_t", (IN_DIM, BCk * J), BF16)
    din("rhs_const", (25, J * G), BF16)
    din("W0p", (28, H), BF16)
    din("WS0p", (28, 8), BF16)
    din("GW", (L, H, H), BF16)
    din("WS", (L, H, 8), BF16)
    din("GB", (L, H), F32)
    din("LNG", (L, H), F32)
    din("LNB", (L, H), F32)
    din("W1", (H, H // 2), BF16)
    din("RW1p", (28, H // 2), BF16)
    din("B1", (H // 2,), F32)
    din("W2p", (H // 2, OUT_DIM), BF16)
    din("C2B2", (2, OUT_DIM), F32)
    io["y"] = dt("y", (BCk, J, OUT_DIM), F32, kind="ExternalOutput").ap()
    with tile.TileContext(nc) as tc:
        with ExitStack() as ctx:
            kernel_body(ctx, tc, io, G2, NB)
    nc.compile()
    _NC_CACHE[key] = nc
    return nc


def make_in_maps(inputs, G2=G2_FULL, NB=NB_FULL):
    import ml_dtypes
    bf = ml_dtypes.bfloat16
    pp = host_prep(inputs)
    G = 128 * G2
    BCk = G * NB
    x = np.asarray(inputs["x"], dtype=np.float32)
    shared = dict(
        rhs_const=np.ascontiguousarray(make_rhs_const(G)),
        W0p=np.ascontiguousarray(pp["W0p"].astype(bf)),
        WS0p=np.ascontiguousarray(pp["WS0p"].astype(bf)),
        GW=np.ascontiguousarray(pp["gat_w"].astype(bf)),
        WS=np.ascontiguousarray(pp["ws"].astype(bf)),
        GB=np.ascontiguousarray(pp["gat_b"].astype(np.float32)),
        LNG=np.ascontiguousarray(pp["ln_g"].astype(np.float32)),
        LNB=np.ascontiguousarray(pp["ln_b"].astype(np.float32)),
        W1=np.ascontiguousarray(pp["w1"].astype(bf)),
        RW1p=np.ascontiguousarray(pp["RW1p"].astype(bf)),
        B1=np.ascontiguousarray(pp["b1"].astype(np.float32)),
        W2p=np.ascontiguousarray(pp["W2p"].astype(bf)),
        C2B2=np.ascontiguousarray(
            np.stack([pp["c2"], pp["b2p"]]).astype(np.float32)),
    )
    in_maps = []
    for core in range(N_CORES):
        xc = x[core * BCk:(core + 1) * BCk]
        m = dict(shared)
        m["x_t"] = np.ascontiguousarray(make_x_t(xc, G))
        in_maps.append(m)
    return in_maps


def run_on_cores(inputs, G2=G2_FULL, NB=NB_FULL, trace=False):
    _install_ntff_shim()
    from concourse.bass_utils import run_bass_kernel_spmd
    nc = _build_nc(G2, NB)
    in_maps = make_in_maps(inputs, G2, NB)
    res = run_bass_kernel_spmd(nc, in_maps, list(range(N_CORES)), trace=trace)
    ys = [res.results[c]["y"] for c in range(N_CORES)]
    y = np.concatenate(ys, axis=0).astype(np.float32)
    return y, res


def kernel(**inputs) -> np.ndarray:
    y, _ = run_on_cores(inputs)
    return y

